# revision 4
# baseline (speedup 1.0000x reference)
"""Trainium2 Bass kernel: quantized BasicBlock (quant-conv3x3 -> bn -> relu ->
quant-conv3x3 -> bn -> +residual -> relu).

Sharding: data-parallel over the batch dim of x across 8 NeuronCores (8 images
per core).  Weight quantization (centroid/deviation pipeline) is replicated on
every core, computed on-device.

Math notes:
  - jnp.round (round-half-even) is implemented with the fp32 magic-number
    trick: rne(v) = (v + 1.5*2^23) - 1.5*2^23 for |v| < 2^22.
  - Quantized weights are integer "levels" dev+cent = k/8 with |k| < 2048,
    exactly representable in fp16.  The global scale `step` is folded into the
    BN scale vector, so matmuls run in fp16 (4x faster than fp32 on the PE)
    with fp32 PSUM accumulation and no weight-precision loss.
"""

import sys

for _p in ("/opt/trn_rl_repo",):
    if _p not in sys.path:
        sys.path.insert(0, _p)

from contextlib import ExitStack

import numpy as np

import concourse.bass as bass
import concourse.tile as tile
from concourse import bacc, bass_isa, mybir
from concourse.bass_utils import run_bass_kernel_spmd
from concourse.masks import make_identity

P = 128
B, C, H, W = 64, 256, 28, 28
NCORES = 8
BPC = B // NCORES          # images per core
CK = C // P                # channel chunks (2)
TAPS = 9
HP, WP = H + 2, W + 2      # zero-padded spatial 30x30
NR = H // 2                # rows per psum chunk (14)
NN = NR * W                # matmul free dim (392)
F32 = mybir.dt.float32
F16 = mybir.dt.float16

MAGIC = 12582912.0         # 1.5 * 2**23  (fp32 RNE round-to-int trick)
HALF_LVLS = 127.0
LV = 8.0                   # 2**(NUM_BITS-1)
CSTEP = HALF_LVLS / LV     # 15.875
DEVW = 0.5 * HALF_LVLS     # 63.5
BN_EPS = 1e-5

AF = mybir.ActivationFunctionType
OP = mybir.AluOpType
AX = mybir.AxisListType


def _emit(nc, tc, ctx, td):
    """Emit the whole per-core program.  td: dict of DRAM tensor handles."""
    const = ctx.enter_context(tc.tile_pool(name="const", bufs=1))
    bnp = ctx.enter_context(tc.tile_pool(name="bnp", bufs=2))
    wbig = ctx.enter_context(tc.tile_pool(name="wbig", bufs=2))
    whalf = ctx.enter_context(tc.tile_pool(name="whalf", bufs=4))
    wqp = ctx.enter_context(tc.tile_pool(name="wqp", bufs=1))
    wtp = ctx.enter_context(tc.tile_pool(name="wtp", bufs=1))
    tpp = ctx.enter_context(tc.tile_pool(name="tpp", bufs=2, space="PSUM"))
    psp = ctx.enter_context(tc.tile_pool(name="psp", bufs=6, space="PSUM"))
    pxf = ctx.enter_context(tc.tile_pool(name="pxf", bufs=3))
    pxp = ctx.enter_context(tc.tile_pool(name="pxp", bufs=3))
    phh = ctx.enter_context(tc.tile_pool(name="phh", bufs=3))
    pyy = ctx.enter_context(tc.tile_pool(name="pyy", bufs=2))
    pep = ctx.enter_context(tc.tile_pool(name="pep", bufs=3))

    ident16 = const.tile([P, P], F16, name="ident16", tag="ident16")
    make_identity(nc, ident16)
    magicv = const.tile([P, 1], F32, name="magicv", tag="magicv")
    nc.gpsimd.memset(magicv[:], MAGIC)

    wT = {}      # wT[j][k] : [P(ci), CK(m), TAPS, P(co)] fp16
    inv_s = {}   # BN scale with quant step folded in: [P, CK]
    bvec = {}    # BN bias: [P, CK]

    # ---------------- image loads ------------------------------------------
    x_view = td["x"].ap().rearrange("b (c p) h w -> b p c h w", p=P)
    y_view = td["y"].ap().rearrange("b (c p) h w -> b p c h w", p=P)
    xf_t = [None] * BPC
    xp_t = [None] * BPC
    h_t = [None] * BPC

    def load_x(i):
        xf = pxf.tile([P, CK, H, W], F32, name=f"xf{i}", tag="xf")
        nc.sync.dma_start(xf[:], x_view[i])
        xp = pxp.tile([P, CK, HP, WP], F16, name=f"xp{i}", tag="xp")
        nc.gpsimd.memset(xp[:], 0.0)
        nc.scalar.copy(xp[:, :, 1 : 1 + H, 1 : 1 + W], xf[:])
        xf_t[i], xp_t[i] = xf, xp

    # ---------------- per-weight quantization ------------------------------
    def quant_weight(j):
        # BN vectors -> [P, CK] tiles  (channel ch = c*128 + p)
        gv = bnp.tile([P, CK], F32, name=f"gv{j}", tag=f"gv{j}")
        bev = bnp.tile([P, CK], F32, name=f"bev{j}", tag=f"bev{j}")
        muv = bnp.tile([P, CK], F32, name=f"muv{j}", tag=f"muv{j}")
        vav = bnp.tile([P, CK], F32, name=f"vav{j}", tag=f"vav{j}")
        nc.sync.dma_start(gv[:], td[f"gamma{j}"].ap().rearrange("(c p) -> p c", p=P))
        nc.sync.dma_start(bev[:], td[f"beta{j}"].ap().rearrange("(c p) -> p c", p=P))
        nc.sync.dma_start(muv[:], td[f"mean{j}"].ap().rearrange("(c p) -> p c", p=P))
        nc.sync.dma_start(vav[:], td[f"var{j}"].ap().rearrange("(c p) -> p c", p=P))

        tv = bnp.tile([P, CK], F32, name=f"tv{j}", tag="btmp")
        nc.vector.tensor_scalar_add(tv[:], vav[:], BN_EPS)
        rv = bnp.tile([P, CK], F32, name=f"rv{j}", tag="btmp")
        nc.vector.reciprocal(rv[:], tv[:])
        sv = bnp.tile([P, CK], F32, name=f"sv{j}", tag="btmp")
        nc.scalar.activation(sv[:], rv[:], AF.Sqrt)           # rsqrt(var+eps)
        inv = bnp.tile([P, CK], F32, name=f"inv{j}", tag=f"inv{j}")
        nc.vector.tensor_mul(inv[:], sv[:], gv[:])            # gamma * rsqrt
        mi = bnp.tile([P, CK], F32, name=f"mi{j}", tag="btmp")
        nc.vector.tensor_mul(mi[:], muv[:], inv[:])
        bv = const.tile([P, CK], F32, name=f"bv{j}", tag=f"bv{j}")
        nc.vector.tensor_sub(bv[:], bev[:], mi[:])            # beta - mean*inv
        bvec[j] = bv

        # weight load in natural [co, ci*3*3] layout (contiguous rows),
        # one DMA per co-chunk so the per-chunk pipelines can start early
        w32 = wbig.tile([P, CK, C, TAPS], F32, name=f"w32_{j}", tag="wbig")
        wsrc = td[f"w{j}"].ap().rearrange("(c p) ci kh kw -> p c ci (kh kw)", p=P)
        for c in range(CK):
            nc.sync.dma_start(w32[:, c], wsrc[:, c])

        # global absmax -> step = max/127, istep = 127/max
        pmh = []
        for c in range(CK):
            ph = bnp.tile([P, 1], F32, name=f"pmh{j}_{c}", tag="pmh")
            nc.vector.tensor_reduce(
                ph[:], w32[:, c], axis=AX.XY, op=OP.max, apply_absolute_value=True
            )
            pmh.append(ph)
        pm = bnp.tile([P, 1], F32, name=f"pm{j}", tag="pm")
        nc.vector.tensor_max(pm[:], pmh[0][:], pmh[1][:])
        pmax = bnp.tile([P, 1], F32, name=f"pmax{j}", tag="pmax")
        nc.gpsimd.partition_all_reduce(pmax[:], pm[:], P, bass_isa.ReduceOp.max)
        step = const.tile([P, 1], F32, name=f"step{j}", tag=f"step{j}")
        nc.vector.tensor_scalar_mul(step[:], pmax[:], 1.0 / HALF_LVLS)
        rmax = bnp.tile([P, 1], F32, name=f"rmax{j}", tag="rmax")
        nc.vector.reciprocal(rmax[:], pmax[:])
        istep = bnp.tile([P, 1], F32, name=f"istep{j}", tag="istep")
        nc.vector.tensor_scalar_mul(istep[:], rmax[:], HALF_LVLS)

        # fold step into BN scale: inv_s = inv * step
        ivs = const.tile([P, CK], F32, name=f"ivs{j}", tag=f"ivs{j}")
        nc.vector.tensor_scalar_mul(ivs[:], inv[:], step[:, 0:1])
        inv_s[j] = ivs

        wq = wqp.tile([P, CK, C, TAPS], F16, name=f"wq{j}", tag=f"wq{j}")
        wT[j] = []
        for k in range(CK):
            wt = wtp.tile([P, CK, TAPS, P], F16, name=f"wT{j}_{k}", tag=f"wT{j}_{k}")
            wT[j].append(wt)

        # per co-chunk pipeline: quantize + transpose (chunk c == psum m chunk)
        for c in range(CK):
            # wl = clip(rne(w * istep), -127, 127)   (scale pass on ScalarE)
            wl = whalf.tile([P, C, TAPS], F32, name=f"wl{j}_{c}", tag="whalf")
            nc.scalar.activation(
                wl[:], w32[:, c], AF.Identity, bias=magicv[:, 0:1], scale=istep[:, 0:1]
            )
            wl2 = whalf.tile([P, C, TAPS], F32, name=f"wl2{j}_{c}", tag="whalf")
            nc.vector.tensor_scalar(
                wl2[:], wl[:], MAGIC, HALF_LVLS, OP.subtract, OP.min
            )
            wl3 = whalf.tile([P, C, TAPS], F32, name=f"wl3{j}_{c}", tag="whalf")
            nc.vector.tensor_scalar_max(wl3[:], wl2[:], -HALF_LVLS)

            # per-grain (co, ci) mean over the 9 taps -> centroid levels
            gm = bnp.tile([P, C], F32, name=f"gm{j}_{c}", tag="gm")
            nc.vector.tensor_reduce(gm[:], wl3[:], axis=AX.X, op=OP.add)
            c1 = bnp.tile([P, C], F32, name=f"c1{j}_{c}", tag="c1")
            nc.vector.tensor_scalar(
                c1[:], gm[:], 1.0 / (TAPS * CSTEP), MAGIC, OP.mult, OP.add
            )
            c2 = bnp.tile([P, C], F32, name=f"c2{j}_{c}", tag="c2")
            nc.vector.tensor_scalar(c2[:], c1[:], MAGIC, LV, OP.subtract, OP.min)
            cent = bnp.tile([P, C], F32, name=f"cent{j}_{c}", tag="cent")
            nc.vector.tensor_scalar(cent[:], c2[:], -LV, CSTEP, OP.max, OP.mult)
            centb = cent.unsqueeze(2).broadcast_to((P, C, TAPS))

            # dev = rne(clip(wl - cent, -63.5, 63.5)); wq = dev + cent
            dv = whalf.tile([P, C, TAPS], F32, name=f"dv{j}_{c}", tag="whalf")
            nc.vector.tensor_sub(dv[:], wl3[:], centb)
            dv2 = whalf.tile([P, C, TAPS], F32, name=f"dv2{j}_{c}", tag="whalf")
            nc.vector.tensor_scalar(dv2[:], dv[:], DEVW, -DEVW, OP.min, OP.max)
            dv3 = whalf.tile([P, C, TAPS], F32, name=f"dv3{j}_{c}", tag="whalf")
            nc.vector.tensor_scalar(dv3[:], dv2[:], MAGIC, MAGIC, OP.add, OP.subtract)
            nc.vector.tensor_add(wq[:, c], dv3[:], centb)

            # PE-transpose the 18 (k, tap) blocks of co-chunk c: [co,ci]->[ci,co]
            m = c
            for k in range(CK):
                for t0 in (0, 4, 8):
                    nb = min(4, TAPS - t0)
                    pst = tpp.tile(
                        [P, nb, P], F16, name=f"pst{j}_{m}_{k}_{t0}", tag="tp"
                    )
                    for dt in range(nb):
                        nc.tensor.transpose(
                            pst[:, dt, :],
                            wq[:, m, k * P : (k + 1) * P, t0 + dt],
                            ident16[:],
                        )
                    nc.scalar.copy(wT[j][k][:, m, t0 : t0 + nb, :], pst[:])

    # ---------------- convolutions -----------------------------------------
    def conv_mms(ps, src16, wTj, m, r0):
        idx = 0
        for k in range(CK):
            for dh in range(3):
                for dw in range(3):
                    t = dh * 3 + dw
                    nc.tensor.matmul(
                        ps[:],
                        wTj[k][:, m, t, :],
                        src16[:, k, r0 + dh : r0 + dh + NR, dw : dw + W],
                        start=(idx == 0),
                        stop=(idx == 2 * TAPS - 1),
                    )
                    idx += 1

    def conv1(i):
        hh = phh.tile([P, CK, HP, WP], F16, name=f"h{i}", tag="h")
        nc.gpsimd.memset(hh[:], 0.0)
        h_t[i] = hh
        for m in range(CK):
            for r in range(2):
                r0 = r * NR
                ps = psp.tile([P, NN], F32, name=f"ps1_{i}_{m}_{r}", tag="ps")
                conv_mms(ps, xp_t[i], wT[1], m, r0)
                nc.scalar.activation(
                    hh[:, m, 1 + r0 : 1 + r0 + NR, 1 : 1 + W],
                    ps.rearrange("p (r w) -> p r w", w=W),
                    AF.Relu,
                    bias=bvec[1][:, m : m + 1],
                    scale=inv_s[1][:, m : m + 1],
                )

    def conv2(i):
        yf = pyy.tile([P, CK, H, W], F32, name=f"y{i}", tag="y")
        for m in range(CK):
            for r in range(2):
                r0 = r * NR
                ps = psp.tile([P, NN], F32, name=f"ps2_{i}_{m}_{r}", tag="ps")
                conv_mms(ps, h_t[i], wT[2], m, r0)
                t2 = pep.tile([P, NN], F32, name=f"t2_{i}_{m}_{r}", tag="t2")
                nc.scalar.activation(
                    t2[:],
                    ps[:],
                    AF.Identity,
                    bias=bvec[2][:, m : m + 1],
                    scale=inv_s[2][:, m : m + 1],
                )
                u = pep.tile([P, NN], F32, name=f"u_{i}_{m}_{r}", tag="u")
                xflat = xf_t[i][:, m, r0 : r0 + NR, :].rearrange("p r w -> p (r w)")
                nc.vector.tensor_add(u[:], t2[:], xflat)
                nc.scalar.activation(
                    yf[:, m, r0 : r0 + NR, :],
                    u.rearrange("p (r w) -> p r w", w=W),
                    AF.Relu,
                )
        nc.sync.dma_start(y_view[i], yf[:])

    # ---------------- emission order (engine priority) ---------------------
    for i in range(BPC):
        load_x(i)
    quant_weight(1)
    conv1(0)
    quant_weight(2)
    conv1(1)
    for i in range(BPC):
        if i + 2 < BPC:
            conv1(i + 2)
        conv2(i)


def build_bass():
    nc = bacc.Bacc(
        "TRN2", target_bir_lowering=False, debug=False, num_devices=NCORES
    )
    td = {}
    td["x"] = nc.dram_tensor("x", (BPC, C, H, W), F32, kind="ExternalInput")
    for j in (1, 2):
        td[f"w{j}"] = nc.dram_tensor(f"w{j}", (C, C, 3, 3), F32, kind="ExternalInput")
        for v in ("gamma", "beta", "mean", "var"):
            td[f"{v}{j}"] = nc.dram_tensor(f"{v}{j}", (C,), F32, kind="ExternalInput")
    td["y"] = nc.dram_tensor("y", (BPC, C, H, W), F32, kind="ExternalOutput")

    with tile.TileContext(nc) as tc:
        with ExitStack() as ctx:
            _emit(nc, tc, ctx, td)
    nc.compile()
    return nc


_NC = None


def _get_nc():
    global _NC
    if _NC is None:
        _NC = build_bass()
    return _NC


def make_in_maps(x, w1, gamma1, beta1, mean1, var1, w2, gamma2, beta2, mean2, var2):
    rep = {
        "w1": w1, "gamma1": gamma1, "beta1": beta1, "mean1": mean1, "var1": var1,
        "w2": w2, "gamma2": gamma2, "beta2": beta2, "mean2": mean2, "var2": var2,
    }
    rep = {k: np.ascontiguousarray(np.asarray(v), dtype=np.float32) for k, v in rep.items()}
    in_maps = []
    for c in range(NCORES):
        m = {"x": np.ascontiguousarray(np.asarray(x)[c * BPC : (c + 1) * BPC], dtype=np.float32)}
        m.update(rep)
        in_maps.append(m)
    return in_maps


def kernel(x, w1, gamma1, beta1, mean1, var1,
           w2, gamma2, beta2, mean2, var2, codebook=None, **_unused):
    nc = _get_nc()
    in_maps = make_in_maps(x, w1, gamma1, beta1, mean1, var1,
                           w2, gamma2, beta2, mean2, var2)
    res = run_bass_kernel_spmd(nc, in_maps, core_ids=list(range(NCORES)))
    return np.concatenate([r["y"] for r in res.results], axis=0)


# revision 5
# speedup vs baseline: 1.2022x; 1.2022x over previous
"""Trainium2 Bass kernel: quantized BasicBlock (quant-conv3x3 -> bn -> relu ->
quant-conv3x3 -> bn -> +residual -> relu).

Sharding: data-parallel over the batch dim of x across 8 NeuronCores (8 images
per core).  Weight quantization (centroid/deviation pipeline) is replicated on
every core, computed on-device.

Math notes:
  - jnp.round (round-half-even) is implemented with the fp32 magic-number
    trick: rne(v) = (v + 1.5*2^23) - 1.5*2^23 for |v| < 2^22.
  - Quantized weights are integer "levels" dev+cent = k/8 with |k| < 2048,
    exactly representable in fp16.  The global scale `step` is folded into the
    BN scale vector, so matmuls run in fp16 (4x faster than fp32 on the PE)
    with fp32 PSUM accumulation and no weight-precision loss.
"""

import sys

for _p in ("/opt/trn_rl_repo",):
    if _p not in sys.path:
        sys.path.insert(0, _p)

from contextlib import ExitStack

import numpy as np

import concourse.bass as bass
import concourse.tile as tile
from concourse import bacc, bass_isa, mybir
from concourse.bass_utils import run_bass_kernel_spmd
from concourse.masks import make_identity

P = 128
B, C, H, W = 64, 256, 28, 28
NCORES = 8
BPC = B // NCORES          # images per core
CK = C // P                # channel chunks (2)
TAPS = 9
HP, WP = H + 2, W + 2      # zero-padded spatial 30x30
NR = H // 2                # rows per psum chunk (14)
NN = NR * W                # matmul free dim (392)
F32 = mybir.dt.float32
F16 = mybir.dt.float16

MAGIC = 12582912.0         # 1.5 * 2**23  (fp32 RNE round-to-int trick)
HALF_LVLS = 127.0
LV = 8.0                   # 2**(NUM_BITS-1)
CSTEP = HALF_LVLS / LV     # 15.875
DEVW = 0.5 * HALF_LVLS     # 63.5
BN_EPS = 1e-5

AF = mybir.ActivationFunctionType
OP = mybir.AluOpType
AX = mybir.AxisListType


def _emit(nc, tc, ctx, td):
    """Emit the whole per-core program.  td: dict of DRAM tensor handles."""
    const = ctx.enter_context(tc.tile_pool(name="const", bufs=1))
    bnp = ctx.enter_context(tc.tile_pool(name="bnp", bufs=2))
    wbig = ctx.enter_context(tc.tile_pool(name="wbig", bufs=1))
    whalf = ctx.enter_context(tc.tile_pool(name="whalf", bufs=3))
    wqp = ctx.enter_context(tc.tile_pool(name="wqp", bufs=1))
    wtp = ctx.enter_context(tc.tile_pool(name="wtp", bufs=1))
    tpp = ctx.enter_context(tc.tile_pool(name="tpp", bufs=2, space="PSUM"))
    psp = ctx.enter_context(tc.tile_pool(name="psp", bufs=6, space="PSUM"))
    pxf = ctx.enter_context(tc.tile_pool(name="pxf", bufs=5))
    pxp = ctx.enter_context(tc.tile_pool(name="pxp", bufs=4))
    phh = ctx.enter_context(tc.tile_pool(name="phh", bufs=3))
    pyy = ctx.enter_context(tc.tile_pool(name="pyy", bufs=2))
    pep = ctx.enter_context(tc.tile_pool(name="pep", bufs=3))

    ident16 = const.tile([P, P], F16, name="ident16", tag="ident16")
    make_identity(nc, ident16)
    magicv = const.tile([P, 1], F32, name="magicv", tag="magicv")
    nc.gpsimd.memset(magicv[:], MAGIC)

    wT = {}      # wT[j][k] : [P(ci), CK(m), TAPS, P(co)] fp16
    inv_s = {}   # BN scale with quant step folded in: [P, CK]
    bvec = {}    # BN bias: [P, CK]

    # ---------------- image loads ------------------------------------------
    x_view = td["x"].ap().rearrange("b (c p) h w -> b p c h w", p=P)
    y_view = td["y"].ap().rearrange("b (c p) h w -> b p c h w", p=P)
    xf_t = [None] * BPC
    xp_t = [None] * BPC
    h_t = [None] * BPC

    def load_x(i):
        xf = pxf.tile([P, CK, H, W], F32, name=f"xf{i}", tag="xf")
        nc.sync.dma_start(xf[:], x_view[i])
        xp = pxp.tile([P, CK, HP, WP], F16, name=f"xp{i}", tag="xp")
        nc.gpsimd.memset(xp[:], 0.0)
        nc.scalar.copy(xp[:, :, 1 : 1 + H, 1 : 1 + W], xf[:])
        xf_t[i], xp_t[i] = xf, xp

    # ---------------- per-weight quantization ------------------------------
    def quant_weight(j):
        # BN vectors -> [P, CK] tiles  (channel ch = c*128 + p)
        gv = bnp.tile([P, CK], F32, name=f"gv{j}", tag=f"gv{j}")
        bev = bnp.tile([P, CK], F32, name=f"bev{j}", tag=f"bev{j}")
        muv = bnp.tile([P, CK], F32, name=f"muv{j}", tag=f"muv{j}")
        vav = bnp.tile([P, CK], F32, name=f"vav{j}", tag=f"vav{j}")
        nc.sync.dma_start(gv[:], td[f"gamma{j}"].ap().rearrange("(c p) -> p c", p=P))
        nc.sync.dma_start(bev[:], td[f"beta{j}"].ap().rearrange("(c p) -> p c", p=P))
        nc.sync.dma_start(muv[:], td[f"mean{j}"].ap().rearrange("(c p) -> p c", p=P))
        nc.sync.dma_start(vav[:], td[f"var{j}"].ap().rearrange("(c p) -> p c", p=P))

        tv = bnp.tile([P, CK], F32, name=f"tv{j}", tag="btmp")
        nc.vector.tensor_scalar_add(tv[:], vav[:], BN_EPS)
        rv = bnp.tile([P, CK], F32, name=f"rv{j}", tag="btmp")
        nc.vector.reciprocal(rv[:], tv[:])
        sv = bnp.tile([P, CK], F32, name=f"sv{j}", tag="btmp")
        nc.scalar.activation(sv[:], rv[:], AF.Sqrt)           # rsqrt(var+eps)
        inv = bnp.tile([P, CK], F32, name=f"inv{j}", tag=f"inv{j}")
        nc.vector.tensor_mul(inv[:], sv[:], gv[:])            # gamma * rsqrt
        mi = bnp.tile([P, CK], F32, name=f"mi{j}", tag="btmp")
        nc.vector.tensor_mul(mi[:], muv[:], inv[:])
        bv = const.tile([P, CK], F32, name=f"bv{j}", tag=f"bv{j}")
        nc.vector.tensor_sub(bv[:], bev[:], mi[:])            # beta - mean*inv
        bvec[j] = bv

        # weight load in natural [co, ci*3*3] layout (contiguous rows),
        # one DMA per co-chunk so the per-chunk pipelines can start early
        w32 = wbig.tile([P, CK, C, TAPS], F32, name=f"w32_{j}", tag="wbig")
        wsrc = td[f"w{j}"].ap().rearrange("(c p) ci kh kw -> p c ci (kh kw)", p=P)
        for c in range(CK):
            nc.sync.dma_start(w32[:, c], wsrc[:, c])

        # global absmax -> step = max/127, istep = 127/max
        pmh = []
        for c in range(CK):
            ph = bnp.tile([P, 1], F32, name=f"pmh{j}_{c}", tag="pmh")
            nc.vector.tensor_reduce(
                ph[:], w32[:, c], axis=AX.XY, op=OP.max, apply_absolute_value=True
            )
            pmh.append(ph)
        pm = bnp.tile([P, 1], F32, name=f"pm{j}", tag="pm")
        nc.vector.tensor_max(pm[:], pmh[0][:], pmh[1][:])
        pmax = bnp.tile([P, 1], F32, name=f"pmax{j}", tag="pmax")
        nc.gpsimd.partition_all_reduce(pmax[:], pm[:], P, bass_isa.ReduceOp.max)
        step = const.tile([P, 1], F32, name=f"step{j}", tag=f"step{j}")
        nc.vector.tensor_scalar_mul(step[:], pmax[:], 1.0 / HALF_LVLS)
        rmax = bnp.tile([P, 1], F32, name=f"rmax{j}", tag="rmax")
        nc.vector.reciprocal(rmax[:], pmax[:])
        istep = bnp.tile([P, 1], F32, name=f"istep{j}", tag="istep")
        nc.vector.tensor_scalar_mul(istep[:], rmax[:], HALF_LVLS)

        # fold step into BN scale: inv_s = inv * step
        ivs = const.tile([P, CK], F32, name=f"ivs{j}", tag=f"ivs{j}")
        nc.vector.tensor_scalar_mul(ivs[:], inv[:], step[:, 0:1])
        inv_s[j] = ivs

        wq = wqp.tile([P, CK, C, TAPS], F16, name=f"wq{j}", tag=f"wq{j}")
        wT[j] = []
        for k in range(CK):
            wt = wtp.tile([P, CK, TAPS, P], F16, name=f"wT{j}_{k}", tag=f"wT{j}_{k}")
            wT[j].append(wt)

        # per co-chunk pipeline: quantize + transpose (chunk c == psum m chunk)
        for c in range(CK):
            # wl = clip(rne(w * istep), -127, 127)   (scale pass on ScalarE)
            wl = whalf.tile([P, C, TAPS], F32, name=f"wl{j}_{c}", tag="whalf")
            nc.scalar.activation(
                wl[:], w32[:, c], AF.Identity, bias=magicv[:, 0:1], scale=istep[:, 0:1]
            )
            wl2 = whalf.tile([P, C, TAPS], F32, name=f"wl2{j}_{c}", tag="whalf")
            nc.vector.tensor_scalar(
                wl2[:], wl[:], MAGIC, HALF_LVLS, OP.subtract, OP.min
            )
            wl3 = whalf.tile([P, C, TAPS], F32, name=f"wl3{j}_{c}", tag="whalf")
            nc.vector.tensor_scalar_max(wl3[:], wl2[:], -HALF_LVLS)

            # per-grain (co, ci) mean over the 9 taps -> centroid levels
            gm = bnp.tile([P, C], F32, name=f"gm{j}_{c}", tag="gm")
            nc.vector.tensor_reduce(gm[:], wl3[:], axis=AX.X, op=OP.add)
            c1 = bnp.tile([P, C], F32, name=f"c1{j}_{c}", tag="c1")
            nc.vector.tensor_scalar(
                c1[:], gm[:], 1.0 / (TAPS * CSTEP), MAGIC, OP.mult, OP.add
            )
            c2 = bnp.tile([P, C], F32, name=f"c2{j}_{c}", tag="c2")
            nc.vector.tensor_scalar(c2[:], c1[:], MAGIC, LV, OP.subtract, OP.min)
            cent = bnp.tile([P, C], F32, name=f"cent{j}_{c}", tag="cent")
            nc.vector.tensor_scalar(cent[:], c2[:], -LV, CSTEP, OP.max, OP.mult)
            centb = cent.unsqueeze(2).broadcast_to((P, C, TAPS))

            # dev = rne(clip(wl - cent, -63.5, 63.5)); wq = dev + cent
            dv = whalf.tile([P, C, TAPS], F32, name=f"dv{j}_{c}", tag="whalf")
            nc.vector.tensor_sub(dv[:], wl3[:], centb)
            dv2 = whalf.tile([P, C, TAPS], F32, name=f"dv2{j}_{c}", tag="whalf")
            nc.vector.tensor_scalar(dv2[:], dv[:], DEVW, -DEVW, OP.min, OP.max)
            dv3 = whalf.tile([P, C, TAPS], F32, name=f"dv3{j}_{c}", tag="whalf")
            nc.vector.tensor_scalar(dv3[:], dv2[:], MAGIC, MAGIC, OP.add, OP.subtract)
            nc.vector.tensor_add(wq[:, c], dv3[:], centb)

            # PE-transpose the 18 (k, tap) blocks of co-chunk c: [co,ci]->[ci,co]
            m = c
            for k in range(CK):
                for t0 in (0, 4, 8):
                    nb = min(4, TAPS - t0)
                    pst = tpp.tile(
                        [P, nb, P], F16, name=f"pst{j}_{m}_{k}_{t0}", tag="tp"
                    )
                    for dt in range(nb):
                        nc.tensor.transpose(
                            pst[:, dt, :],
                            wq[:, m, k * P : (k + 1) * P, t0 + dt],
                            ident16[:],
                        )
                    nc.scalar.copy(wT[j][k][:, m, t0 : t0 + nb, :], pst[:])

    # ---------------- convolutions -----------------------------------------
    def conv_mms(ps, src16, wTj, m, r0):
        idx = 0
        for k in range(CK):
            for dh in range(3):
                for dw in range(3):
                    t = dh * 3 + dw
                    nc.tensor.matmul(
                        ps[:],
                        wTj[k][:, m, t, :],
                        src16[:, k, r0 + dh : r0 + dh + NR, dw : dw + W],
                        start=(idx == 0),
                        stop=(idx == 2 * TAPS - 1),
                    )
                    idx += 1

    def conv1(i):
        hh = phh.tile([P, CK, HP, WP], F16, name=f"h{i}", tag="h")
        nc.gpsimd.memset(hh[:], 0.0)
        h_t[i] = hh
        for m in range(CK):
            for r in range(2):
                r0 = r * NR
                ps = psp.tile([P, NN], F32, name=f"ps1_{i}_{m}_{r}", tag="ps")
                conv_mms(ps, xp_t[i], wT[1], m, r0)
                nc.scalar.activation(
                    hh[:, m, 1 + r0 : 1 + r0 + NR, 1 : 1 + W],
                    ps.rearrange("p (r w) -> p r w", w=W),
                    AF.Relu,
                    bias=bvec[1][:, m : m + 1],
                    scale=inv_s[1][:, m : m + 1],
                )

    def conv2(i):
        yf = pyy.tile([P, CK, H, W], F32, name=f"y{i}", tag="y")
        for m in range(CK):
            for r in range(2):
                r0 = r * NR
                ps = psp.tile([P, NN], F32, name=f"ps2_{i}_{m}_{r}", tag="ps")
                conv_mms(ps, h_t[i], wT[2], m, r0)
                t2 = pep.tile([P, NN], F32, name=f"t2_{i}_{m}_{r}", tag="t2")
                nc.scalar.activation(
                    t2[:],
                    ps[:],
                    AF.Identity,
                    bias=bvec[2][:, m : m + 1],
                    scale=inv_s[2][:, m : m + 1],
                )
                u = pep.tile([P, NN], F32, name=f"u_{i}_{m}_{r}", tag="u")
                xflat = xf_t[i][:, m, r0 : r0 + NR, :].rearrange("p r w -> p (r w)")
                nc.vector.tensor_add(u[:], t2[:], xflat)
                nc.scalar.activation(
                    yf[:, m, r0 : r0 + NR, :],
                    u.rearrange("p (r w) -> p r w", w=W),
                    AF.Relu,
                )
        nc.sync.dma_start(y_view[i], yf[:])

    # ---------------- emission order (engine priority) ---------------------
    for i in range(BPC):
        load_x(i)
    quant_weight(1)
    conv1(0)
    quant_weight(2)
    conv1(1)
    for i in range(BPC):
        if i + 2 < BPC:
            conv1(i + 2)
        conv2(i)


def build_bass():
    nc = bacc.Bacc(
        "TRN2", target_bir_lowering=False, debug=False, num_devices=NCORES
    )
    td = {}
    td["x"] = nc.dram_tensor("x", (BPC, C, H, W), F32, kind="ExternalInput")
    for j in (1, 2):
        td[f"w{j}"] = nc.dram_tensor(f"w{j}", (C, C, 3, 3), F32, kind="ExternalInput")
        for v in ("gamma", "beta", "mean", "var"):
            td[f"{v}{j}"] = nc.dram_tensor(f"{v}{j}", (C,), F32, kind="ExternalInput")
    td["y"] = nc.dram_tensor("y", (BPC, C, H, W), F32, kind="ExternalOutput")

    with tile.TileContext(nc) as tc:
        with ExitStack() as ctx:
            _emit(nc, tc, ctx, td)
    nc.compile()
    return nc


_NC = None


def _get_nc():
    global _NC
    if _NC is None:
        _NC = build_bass()
    return _NC


def make_in_maps(x, w1, gamma1, beta1, mean1, var1, w2, gamma2, beta2, mean2, var2):
    rep = {
        "w1": w1, "gamma1": gamma1, "beta1": beta1, "mean1": mean1, "var1": var1,
        "w2": w2, "gamma2": gamma2, "beta2": beta2, "mean2": mean2, "var2": var2,
    }
    rep = {k: np.ascontiguousarray(np.asarray(v), dtype=np.float32) for k, v in rep.items()}
    in_maps = []
    for c in range(NCORES):
        m = {"x": np.ascontiguousarray(np.asarray(x)[c * BPC : (c + 1) * BPC], dtype=np.float32)}
        m.update(rep)
        in_maps.append(m)
    return in_maps


def kernel(x, w1, gamma1, beta1, mean1, var1,
           w2, gamma2, beta2, mean2, var2, codebook=None, **_unused):
    nc = _get_nc()
    in_maps = make_in_maps(x, w1, gamma1, beta1, mean1, var1,
                           w2, gamma2, beta2, mean2, var2)
    res = run_bass_kernel_spmd(nc, in_maps, core_ids=list(range(NCORES)))
    return np.concatenate([r["y"] for r in res.results], axis=0)


# revision 8
# speedup vs baseline: 1.2323x; 1.0251x over previous
"""Trainium2 Bass kernel: quantized BasicBlock (quant-conv3x3 -> bn -> relu ->
quant-conv3x3 -> bn -> +residual -> relu).

Sharding: data-parallel over the batch dim of x across 8 NeuronCores (8 images
per core).  Weight quantization (centroid/deviation pipeline) is replicated on
every core, computed on-device.

Math notes:
  - jnp.round (round-half-even) is implemented with the fp32 magic-number
    trick: rne(v) = (v + 1.5*2^23) - 1.5*2^23 for |v| < 2^22.
  - Quantized weights are integer "levels" dev+cent = k/8 with |k| < 2048,
    exactly representable in fp16.  The global scale `step` is folded into the
    BN scale vector, so matmuls run in fp16 (4x faster than fp32 on the PE)
    with fp32 PSUM accumulation and no weight-precision loss.
"""

import sys

for _p in ("/opt/trn_rl_repo",):
    if _p not in sys.path:
        sys.path.insert(0, _p)

from contextlib import ExitStack

import numpy as np

import concourse.bass as bass
import concourse.tile as tile
from concourse import bacc, bass_isa, mybir
from concourse.bass_utils import run_bass_kernel_spmd
from concourse.masks import make_identity

P = 128
B, C, H, W = 64, 256, 28, 28
NCORES = 8
BPC = B // NCORES          # images per core
CK = C // P                # channel chunks (2)
TAPS = 9
HP, WP = H + 2, W + 2      # zero-padded spatial 30x30
NR = H // 2                # rows per psum chunk (14)
NN = NR * W                # matmul free dim (392)
F32 = mybir.dt.float32
F16 = mybir.dt.float16

MAGIC = 12582912.0         # 1.5 * 2**23  (fp32 RNE round-to-int trick)
HALF_LVLS = 127.0
LV = 8.0                   # 2**(NUM_BITS-1)
CSTEP = HALF_LVLS / LV     # 15.875
DEVW = 0.5 * HALF_LVLS     # 63.5
BN_EPS = 1e-5

AF = mybir.ActivationFunctionType
OP = mybir.AluOpType
AX = mybir.AxisListType


def _emit(nc, tc, ctx, td):
    """Emit the whole per-core program.  td: dict of DRAM tensor handles."""
    const = ctx.enter_context(tc.tile_pool(name="const", bufs=1))
    bnp = ctx.enter_context(tc.tile_pool(name="bnp", bufs=2))
    wbig = ctx.enter_context(tc.tile_pool(name="wbig", bufs=1))
    whalf = ctx.enter_context(tc.tile_pool(name="whalf", bufs=3))
    wqp = ctx.enter_context(tc.tile_pool(name="wqp", bufs=1))
    wtp = ctx.enter_context(tc.tile_pool(name="wtp", bufs=1))
    tpp = ctx.enter_context(tc.tile_pool(name="tpp", bufs=2, space="PSUM"))
    psp = ctx.enter_context(tc.tile_pool(name="psp", bufs=6, space="PSUM"))
    pxf = ctx.enter_context(tc.tile_pool(name="pxf", bufs=6))
    pxp = ctx.enter_context(tc.tile_pool(name="pxp", bufs=5))
    phh = ctx.enter_context(tc.tile_pool(name="phh", bufs=5))
    pyy = ctx.enter_context(tc.tile_pool(name="pyy", bufs=3))
    pep = ctx.enter_context(tc.tile_pool(name="pep", bufs=3))

    ident16 = const.tile([P, P], F16, name="ident16", tag="ident16")
    make_identity(nc, ident16)
    magicv = const.tile([P, 1], F32, name="magicv", tag="magicv")
    nc.gpsimd.memset(magicv[:], MAGIC)

    wT = {}      # wT[j][k] : [P(ci), CK(m), TAPS, P(co)] fp16
    inv_s = {}   # BN scale with quant step folded in: [P, CK]
    bvec = {}    # BN bias: [P, CK]
    _w32 = {}
    _wq = {}
    _istep = {}

    # ---------------- image loads ------------------------------------------
    x_view = td["x"].ap().rearrange("b (c p) h w -> b p c h w", p=P)
    y_view = td["y"].ap().rearrange("b (c p) h w -> b p c h w", p=P)
    xf_t = [None] * BPC
    xp_t = [None] * BPC
    h_t = [None] * BPC

    def load_x(i):
        xf = pxf.tile([P, CK, H, W], F32, name=f"xf{i}", tag="xf")
        nc.sync.dma_start(xf[:], x_view[i])
        xp = pxp.tile([P, CK, HP, WP], F16, name=f"xp{i}", tag="xp")
        nc.vector.memset(xp[:], 0.0)
        nc.scalar.copy(xp[:, :, 1 : 1 + H, 1 : 1 + W], xf[:])
        xf_t[i], xp_t[i] = xf, xp

    # ---------------- per-weight quantization ------------------------------
    def quant_load(j):
        """DMAs for weight j + BN vector prep (cheap)."""
        # weight load in natural [co, ci*3*3] layout (contiguous rows)
        w32 = wbig.tile([P, CK, C, TAPS], F32, name=f"w32_{j}", tag="wbig")
        wsrc = td[f"w{j}"].ap().rearrange("(c p) ci kh kw -> p c ci (kh kw)", p=P)
        for c in range(CK):
            nc.sync.dma_start(w32[:, c], wsrc[:, c])
        _w32[j] = w32

        # BN vectors -> [P, CK] tiles  (channel ch = c*128 + p)
        gv = bnp.tile([P, CK], F32, name=f"gv{j}", tag=f"gv{j}")
        bev = bnp.tile([P, CK], F32, name=f"bev{j}", tag=f"bev{j}")
        muv = bnp.tile([P, CK], F32, name=f"muv{j}", tag=f"muv{j}")
        vav = bnp.tile([P, CK], F32, name=f"vav{j}", tag=f"vav{j}")
        nc.sync.dma_start(gv[:], td[f"gamma{j}"].ap().rearrange("(c p) -> p c", p=P))
        nc.sync.dma_start(bev[:], td[f"beta{j}"].ap().rearrange("(c p) -> p c", p=P))
        nc.sync.dma_start(muv[:], td[f"mean{j}"].ap().rearrange("(c p) -> p c", p=P))
        nc.sync.dma_start(vav[:], td[f"var{j}"].ap().rearrange("(c p) -> p c", p=P))

        tv = bnp.tile([P, CK], F32, name=f"tv{j}", tag="btmp")
        nc.vector.tensor_scalar_add(tv[:], vav[:], BN_EPS)
        rv = bnp.tile([P, CK], F32, name=f"rv{j}", tag="btmp")
        nc.vector.reciprocal(rv[:], tv[:])
        sv = bnp.tile([P, CK], F32, name=f"sv{j}", tag="btmp")
        nc.scalar.activation(sv[:], rv[:], AF.Sqrt)           # rsqrt(var+eps)
        inv = bnp.tile([P, CK], F32, name=f"inv{j}", tag=f"inv{j}")
        nc.vector.tensor_mul(inv[:], sv[:], gv[:])            # gamma * rsqrt
        mi = bnp.tile([P, CK], F32, name=f"mi{j}", tag="btmp")
        nc.vector.tensor_mul(mi[:], muv[:], inv[:])
        bv = const.tile([P, CK], F32, name=f"bv{j}", tag=f"bv{j}")
        nc.vector.tensor_sub(bv[:], bev[:], mi[:])            # beta - mean*inv
        bvec[j] = bv

        # global absmax -> step = max/127, istep = 127/max
        pmh = []
        for c in range(CK):
            ph = bnp.tile([P, 1], F32, name=f"pmh{j}_{c}", tag="pmh")
            nc.vector.tensor_reduce(
                ph[:], w32[:, c], axis=AX.XY, op=OP.max, apply_absolute_value=True
            )
            pmh.append(ph)
        pm = bnp.tile([P, 1], F32, name=f"pm{j}", tag="pm")
        nc.vector.tensor_max(pm[:], pmh[0][:], pmh[1][:])
        pmax = bnp.tile([P, 1], F32, name=f"pmax{j}", tag="pmax")
        nc.gpsimd.partition_all_reduce(pmax[:], pm[:], P, bass_isa.ReduceOp.max)
        step = const.tile([P, 1], F32, name=f"step{j}", tag=f"step{j}")
        nc.vector.tensor_scalar_mul(step[:], pmax[:], 1.0 / HALF_LVLS)
        rmax = bnp.tile([P, 1], F32, name=f"rmax{j}", tag="rmax")
        nc.vector.reciprocal(rmax[:], pmax[:])
        istep = const.tile([P, 1], F32, name=f"istep{j}", tag=f"istep{j}")
        nc.vector.tensor_scalar_mul(istep[:], rmax[:], HALF_LVLS)
        _istep[j] = istep

        # fold step into BN scale: inv_s = inv * step
        ivs = const.tile([P, CK], F32, name=f"ivs{j}", tag=f"ivs{j}")
        nc.vector.tensor_scalar_mul(ivs[:], inv[:], step[:, 0:1])
        inv_s[j] = ivs

        wq = wqp.tile([P, CK, C, TAPS], F16, name=f"wq{j}", tag=f"wq{j}")
        _wq[j] = wq
        wT[j] = []
        for k in range(CK):
            wt = wtp.tile([P, CK, TAPS, P], F16, name=f"wT{j}_{k}", tag=f"wT{j}_{k}")
            wT[j].append(wt)

    def quant_chain(j):
        """Quantization pipeline at (co-chunk, ci-half) granularity so the
        first transposes (and conv matmuls) start as early as possible."""
        w32, wq, istep = _w32[j], _wq[j], _istep[j]
        for c in range(CK):      # co-chunk == psum m chunk
            for k in range(CK):  # ci half (128 input channels)
                ks = slice(k * P, (k + 1) * P)
                src = w32[:, c, ks, :]
                # wl = rne(w * istep) (+MAGIC on ScalarE, -MAGIC merged below)
                wl = whalf.tile([P, P, TAPS], F32, name=f"wl{j}_{c}_{k}", tag="wh")
                nc.scalar.activation(
                    wl[:], src, AF.Identity, bias=magicv[:, 0:1], scale=istep[:, 0:1]
                )
                wl2 = whalf.tile([P, P, TAPS], F32, name=f"wl2{j}_{c}_{k}", tag="wh")
                nc.vector.tensor_scalar(
                    wl2[:], wl[:], MAGIC, HALF_LVLS, OP.subtract, OP.min
                )
                wl3 = whalf.tile([P, P, TAPS], F32, name=f"wl3{j}_{c}_{k}", tag="wh")
                nc.vector.tensor_scalar_max(wl3[:], wl2[:], -HALF_LVLS)

                # per-grain (co, ci) mean over the 9 taps -> centroid levels
                gm = bnp.tile([P, P], F32, name=f"gm{j}_{c}_{k}", tag="gm")
                nc.vector.tensor_reduce(gm[:], wl3[:], axis=AX.X, op=OP.add)
                c1 = bnp.tile([P, P], F32, name=f"c1{j}_{c}_{k}", tag="c1")
                nc.vector.tensor_scalar(
                    c1[:], gm[:], 1.0 / (TAPS * CSTEP), MAGIC, OP.mult, OP.add
                )
                c2 = bnp.tile([P, P], F32, name=f"c2{j}_{c}_{k}", tag="c2")
                nc.vector.tensor_scalar(c2[:], c1[:], MAGIC, LV, OP.subtract, OP.min)
                cent = bnp.tile([P, P], F32, name=f"cent{j}_{c}_{k}", tag="cent")
                nc.vector.tensor_scalar(cent[:], c2[:], -LV, CSTEP, OP.max, OP.mult)
                centb = cent.unsqueeze(2).broadcast_to((P, P, TAPS))

                # dev = rne(clip(wl - cent, -63.5, 63.5)); wq = dev + cent
                dv = whalf.tile([P, P, TAPS], F32, name=f"dv{j}_{c}_{k}", tag="wh")
                nc.vector.tensor_sub(dv[:], wl3[:], centb)
                dv2 = whalf.tile([P, P, TAPS], F32, name=f"dv2{j}_{c}_{k}", tag="wh")
                nc.vector.tensor_scalar(dv2[:], dv[:], DEVW, -DEVW, OP.min, OP.max)
                dv3 = whalf.tile([P, P, TAPS], F32, name=f"dv3{j}_{c}_{k}", tag="wh")
                nc.vector.tensor_scalar(
                    dv3[:], dv2[:], MAGIC, MAGIC, OP.add, OP.subtract
                )
                nc.vector.tensor_add(wq[:, c, ks, :], dv3[:], centb)

                # PE-transpose the 9 taps of this (m=c, k): [co,ci] -> [ci,co]
                m = c
                for t0 in (0, 4, 8):
                    nb = min(4, TAPS - t0)
                    pst = tpp.tile(
                        [P, nb, P], F16, name=f"pst{j}_{m}_{k}_{t0}", tag="tp"
                    )
                    for dt in range(nb):
                        nc.tensor.transpose(
                            pst[:, dt, :],
                            wq[:, m, k * P : (k + 1) * P, t0 + dt],
                            ident16[:],
                        )
                    nc.scalar.copy(wT[j][k][:, m, t0 : t0 + nb, :], pst[:])

    # ---------------- convolutions -----------------------------------------
    def conv_mms(ps, src16, wTj, m, r0):
        idx = 0
        for k in range(CK):
            for dh in range(3):
                for dw in range(3):
                    t = dh * 3 + dw
                    nc.tensor.matmul(
                        ps[:],
                        wTj[k][:, m, t, :],
                        src16[:, k, r0 + dh : r0 + dh + NR, dw : dw + W],
                        start=(idx == 0),
                        stop=(idx == 2 * TAPS - 1),
                    )
                    idx += 1

    def conv1(i):
        hh = phh.tile([P, CK, HP, WP], F16, name=f"h{i}", tag="h")
        nc.vector.memset(hh[:], 0.0)
        h_t[i] = hh
        for m in range(CK):
            for r in range(2):
                r0 = r * NR
                ps = psp.tile([P, NN], F32, name=f"ps1_{i}_{m}_{r}", tag="ps")
                conv_mms(ps, xp_t[i], wT[1], m, r0)
                nc.scalar.activation(
                    hh[:, m, 1 + r0 : 1 + r0 + NR, 1 : 1 + W],
                    ps.rearrange("p (r w) -> p r w", w=W),
                    AF.Relu,
                    bias=bvec[1][:, m : m + 1],
                    scale=inv_s[1][:, m : m + 1],
                )

    def conv2(i):
        for m in range(CK):
            yf = pyy.tile([P, H, W], F32, name=f"y{i}_{m}", tag="y")
            for r in range(2):
                r0 = r * NR
                ps = psp.tile([P, NN], F32, name=f"ps2_{i}_{m}_{r}", tag="ps")
                conv_mms(ps, h_t[i], wT[2], m, r0)
                t2 = pep.tile([P, NN], F32, name=f"t2_{i}_{m}_{r}", tag="t2")
                nc.scalar.activation(
                    t2[:],
                    ps[:],
                    AF.Identity,
                    bias=bvec[2][:, m : m + 1],
                    scale=inv_s[2][:, m : m + 1],
                )
                u = pep.tile([P, NN], F32, name=f"u_{i}_{m}_{r}", tag="u")
                xflat = xf_t[i][:, m, r0 : r0 + NR, :].rearrange("p r w -> p (r w)")
                nc.vector.tensor_add(u[:], t2[:], xflat)
                nc.scalar.activation(
                    yf[:, r0 : r0 + NR, :],
                    u.rearrange("p (r w) -> p r w", w=W),
                    AF.Relu,
                )
            nc.gpsimd.dma_start(y_view[i][:, m], yf[:])

    # ---------------- emission order (engine priority) ---------------------
    quant_load(1)
    for i in range(3):
        load_x(i)
    quant_load(2)
    for i in range(3, BPC):
        load_x(i)
    quant_chain(1)
    conv1(0)
    conv1(1)
    quant_chain(2)
    conv1(2)
    conv1(3)
    for i in range(BPC):
        if i + 4 < BPC:
            conv1(i + 4)
        conv2(i)


def build_bass():
    nc = bacc.Bacc(
        "TRN2", target_bir_lowering=False, debug=False, num_devices=NCORES
    )
    td = {}
    td["x"] = nc.dram_tensor("x", (BPC, C, H, W), F32, kind="ExternalInput")
    for j in (1, 2):
        td[f"w{j}"] = nc.dram_tensor(f"w{j}", (C, C, 3, 3), F32, kind="ExternalInput")
        for v in ("gamma", "beta", "mean", "var"):
            td[f"{v}{j}"] = nc.dram_tensor(f"{v}{j}", (C,), F32, kind="ExternalInput")
    td["y"] = nc.dram_tensor("y", (BPC, C, H, W), F32, kind="ExternalOutput")

    with tile.TileContext(nc) as tc:
        with ExitStack() as ctx:
            _emit(nc, tc, ctx, td)
    nc.compile()
    return nc


_NC = None


def _get_nc():
    global _NC
    if _NC is None:
        _NC = build_bass()
    return _NC


def make_in_maps(x, w1, gamma1, beta1, mean1, var1, w2, gamma2, beta2, mean2, var2):
    rep = {
        "w1": w1, "gamma1": gamma1, "beta1": beta1, "mean1": mean1, "var1": var1,
        "w2": w2, "gamma2": gamma2, "beta2": beta2, "mean2": mean2, "var2": var2,
    }
    rep = {k: np.ascontiguousarray(np.asarray(v), dtype=np.float32) for k, v in rep.items()}
    in_maps = []
    for c in range(NCORES):
        m = {"x": np.ascontiguousarray(np.asarray(x)[c * BPC : (c + 1) * BPC], dtype=np.float32)}
        m.update(rep)
        in_maps.append(m)
    return in_maps


def kernel(x, w1, gamma1, beta1, mean1, var1,
           w2, gamma2, beta2, mean2, var2, codebook=None, **_unused):
    nc = _get_nc()
    in_maps = make_in_maps(x, w1, gamma1, beta1, mean1, var1,
                           w2, gamma2, beta2, mean2, var2)
    res = run_bass_kernel_spmd(nc, in_maps, core_ids=list(range(NCORES)))
    return np.concatenate([r["y"] for r in res.results], axis=0)
